# revision 18
# baseline (speedup 1.0000x reference)
"""AttentiveLSTM Trainium2 kernel.

Strategy: data-parallel over batch (B=100 -> 104 = 8 cores x 13), all
activations kept H-major ("transposed", H on partitions as 6 chunks of 128)
so the recurrent matmuls run weight-stationary (lhsT = natural (H_in, H_out)
weight tiles) and LayerNorm statistics are computed with ones-vector matmuls
on the tensor engine.  LN affine params and biases are folded into the
weights on the host.  Attention over the 2-entry KV set collapses to
p = sigmoid(scale * q . (k_h - k_e));  ctx = v_e + p * (v_h - v_e),
with per-head dots done via constant mask matmuls.  The embedding+LN path
(kv_e) is precomputed on-device for all timesteps before the scan.
"""

import numpy as np
import ml_dtypes
from contextlib import ExitStack

import concourse.bass as bass
import concourse.bacc as bacc
import concourse.tile as tile
from concourse import mybir
from concourse.bass import ds
from concourse.bass_utils import run_bass_kernel_spmd

F32 = mybir.dt.float32
BF16 = mybir.dt.bfloat16

H = 768
NH = 12
HD = 64
V = 50257
B = 100
T = 256
EPS = 1e-5
NCORES = 8
BL = 13          # batch rows per core (padded 104)
CH = 6           # H / 128
ROWS = T * BL    # 3328 rows per core in the kv_e precompute
SCALE = 1.0 / np.sqrt(HD)

bf16 = ml_dtypes.bfloat16


def _ap_bcast(t, part, offset_elems, dims):
    """Build a broadcast AP view over tile `t` (an AP): partition dim from t,
    free dims given as (step, count) pairs."""
    return bass.AP(
        tensor=t.tensor,
        offset=t.offset + offset_elems,
        ap=[list(t.ap[0])] + [[s, c] for (s, c) in dims],
    )


def build_bass(T_steps=T, unroll_static=False, passes=1, debug_step=None):
    nc = bacc.Bacc("TRN2", target_bir_lowering=False)
    rows = T_steps * BL
    dbg = {}
    if debug_step is not None:
        for nm, shp, dt_ in [("dbg_hn", [128, CH * BL], BF16), ("dbg_d", [128, CH * BL], BF16),
                             ("dbg_m", [128, CH * BL], BF16), ("dbg_p", [NH, BL], BF16),
                             ("dbg_ctx", [128, CH * BL], BF16), ("dbg_hx", [128, CH * BL], F32),
                             ("dbg_xn", [128, CH * BL], BF16), ("dbg_g", [128, 4 * CH * BL], F32),
                             ("dbg_sb", [1, 2 * BL], F32), ("dbg_u", [128, CH * BL], F32)]:
            dbg[nm] = nc.dram_tensor(nm, shp, dt_, kind="ExternalOutput")

    # ---------------- DRAM I/O ----------------
    e2T_d = nc.dram_tensor("e2T", [128, CH * rows], BF16, kind="ExternalInput")
    w1_d = nc.dram_tensor("w1", [128, CH * 3 * H], BF16, kind="ExternalInput")
    wc_d = nc.dram_tensor("wc", [128, CH * H], BF16, kind="ExternalInput")
    whh_d = nc.dram_tensor("whh", [128, CH * 4 * H], BF16, kind="ExternalInput")
    vbias_d = nc.dram_tensor("vbias", [1, H], BF16, kind="ExternalInput")
    qbias_d = nc.dram_tensor("qbias", [1, H], BF16, kind="ExternalInput")
    cbias_d = nc.dram_tensor("cbias", [1, H], BF16, kind="ExternalInput")
    hbias_d = nc.dram_tensor("hbias", [128, 4 * CH], F32, kind="ExternalInput")
    maskS_d = nc.dram_tensor("maskS", [128, CH * NH], BF16, kind="ExternalInput")
    maskB_d = nc.dram_tensor("maskB", [NH, CH * 128], BF16, kind="ExternalInput")
    out_d = nc.dram_tensor("out", [128, T_steps * CH * BL], F32, kind="ExternalOutput")

    with ExitStack() as top:
        tc = top.enter_context(tile.TileContext(nc))
        persist = top.enter_context(tc.tile_pool(name="persist", bufs=1))

        # persistent SBUF tensors
        kv = persist.tile([128, 2, CH, rows], BF16)          # k_e (no bias), v_e (biased)
        w1s = persist.tile([128, CH, 3 * H], BF16)
        wcs = persist.tile([128, CH, H], BF16)
        whhs = persist.tile([128, CH, 4 * H], BF16)
        vbias = persist.tile([1, H], BF16)
        ones512 = persist.tile([1, 512], BF16)
        nc.vector.memset(ones512, 1.0)
        qbias = persist.tile([1, H], BF16)
        cbias = persist.tile([1, H], BF16)
        hbias = persist.tile([128, 4 * CH], F32)
        maskS = persist.tile([128, CH, NH], BF16)
        maskB = persist.tile([NH, CH * 128], BF16)
        onescol = persist.tile([128, 1], F32)
        ones128 = persist.tile([1, 128], F32)
        ones13 = persist.tile([1, BL], BF16)
        epsT = persist.tile([1, 1], F32)
        h0 = persist.tile([128, 2, CH, BL], F32)
        c0 = persist.tile([128, CH, BL], F32)
        h1 = persist.tile([128, 2, CH, BL], F32)
        c1 = persist.tile([128, CH, BL], F32)

        nc.vector.memset(onescol, 1.0)
        nc.vector.memset(ones128, 1.0)
        nc.vector.memset(ones13, 1.0)
        nc.vector.memset(epsT, EPS)
        nc.vector.memset(h0, 0.0)
        nc.vector.memset(c0, 0.0)

        nc.sync.dma_start(w1s.rearrange("p c x -> p (c x)"), w1_d[:])
        nc.sync.dma_start(wcs.rearrange("p c x -> p (c x)"), wc_d[:])
        nc.sync.dma_start(whhs.rearrange("p c x -> p (c x)"), whh_d[:])
        nc.sync.dma_start(vbias, vbias_d[:])
        nc.sync.dma_start(qbias, qbias_d[:])
        nc.sync.dma_start(cbias, cbias_d[:])
        nc.sync.dma_start(hbias, hbias_d[:])
        nc.sync.dma_start(maskS.rearrange("p c x -> p (c x)"), maskS_d[:])
        nc.sync.dma_start(maskB, maskB_d[:])

        # ---------------- phase 0: kv_e = LN1(LNe(emb[ids])) @ [Wk|Wv] ----------------
        with ExitStack() as ph0:
            p0 = ph0.enter_context(tc.tile_pool(name="ph0", bufs=1))
            p0ps = ph0.enter_context(tc.tile_pool(name="ph0ps", bufs=1, space="PSUM"))
            HALF = min(rows, 1664)
            e2T = p0.tile([128, CH, HALF], BF16, tag="e2T", bufs=1)
            GN = 512
            for h0_ in range(0, rows, HALF):
                hn_ = min(HALF, rows - h0_)
                # load this half of e2T (chunk-strided in DRAM)
                for c in range(CH):
                    nc.sync.dma_start(e2T[:, c, :hn_],
                                      e2T_d[:, c * rows + h0_: c * rows + h0_ + hn_])
                for g in range((hn_ + GN - 1) // GN):
                    r0 = g * GN
                    n = min(GN, hn_ - r0)
                    for m in range(12):   # W1 out-chunks 6..17 (k then v)
                        mm = m + 6
                        is_v = m >= 6
                        pkv = p0ps.tile([128, GN], F32, tag="pkv", bufs=4)
                        for c in range(CH):
                            nc.tensor.matmul(
                                pkv[:, :n],
                                lhsT=w1s[:, c, mm * 128:(mm + 1) * 128],
                                rhs=e2T[:, c, r0:r0 + n],
                                start=(c == 0), stop=(c == CH - 1 and not is_v),
                            )
                        if not is_v:  # k: no bias, DVE copy
                            nc.vector.tensor_copy(
                                kv[:, 0, m, h0_ + r0:h0_ + r0 + n], pkv[:, :n])
                        else:         # v: bias folded in as a K=1 matmul
                            nc.tensor.matmul(
                                pkv[:, :n],
                                lhsT=vbias[:, (m - 6) * 128:(m - 5) * 128],
                                rhs=ones512[:, :n], start=False, stop=True)
                            nc.scalar.copy(
                                kv[:, 1, m - 6, h0_ + r0:h0_ + r0 + n], pkv[:, :n])

        # ---------------- the recurrent scan ----------------
        sp = top.enter_context(tc.tile_pool(name="scan", bufs=1))
        psB = top.enter_context(tc.tile_pool(name="psB", bufs=2, space="PSUM"))
        psS = top.enter_context(tc.tile_pool(name="psS", bufs=3, space="PSUM"))

        def layer_norm(x2_in, out_bf, tagp):
            """out_bf = bf16 normalized x2_in[:,0] ((128,2,CH,BL) fp32; [:,1] is scratch)."""
            x_in = x2_in[:, 0]
            nc.scalar.activation(x2_in[:, 1], x_in, mybir.ActivationFunctionType.Square)
            stats = psS.tile([1, 2, BL], F32, tag="stats", bufs=1)
            for c in range(CH):
                nc.tensor.matmul(stats[:, :, :], lhsT=onescol, rhs=x2_in[:, :, c, :],
                                 start=(c == 0), stop=(c == CH - 1))
            stats = stats.rearrange("p s b -> p (s b)")
            tb = sp.tile([1, 2 * BL], F32, tag="tb" + tagp)
            nc.vector.tensor_scalar_mul(tb, stats, 1.0 / H)   # [mean | E x^2]
            m2 = sp.tile([1, BL], F32, tag="m2" + tagp)
            nc.vector.tensor_mul(m2, tb[:, 0:BL], tb[:, 0:BL])
            var = sp.tile([1, BL], F32, tag="var" + tagp)
            nc.vector.tensor_sub(var, tb[:, BL:2 * BL], m2)
            std = sp.tile([1, BL], F32, tag="std" + tagp)
            nc.scalar.activation(std, var, mybir.ActivationFunctionType.Sqrt,
                                 bias=epsT[:, 0:1], scale=1.0)
            sb = sp.tile([1, 2 * BL], F32, tag="sb" + tagp)   # [mean*rstd | rstd]
            nc.vector.reciprocal(sb[:, BL:2 * BL], std)
            nc.vector.tensor_mul(sb[:, 0:BL], tb[:, 0:BL], sb[:, BL:2 * BL])
            bc = psS.tile([128, 2 * BL], F32, tag="bc", bufs=1)
            nc.tensor.matmul(bc, lhsT=ones128, rhs=sb, start=True, stop=True)
            rstd_b = _ap_bcast(bc, 128, BL, [(0, CH), (1, BL)])
            mr_b = _ap_bcast(bc, 128, 0, [(0, CH), (1, BL)])
            hh = sp.tile([128, CH, BL], F32, tag="hh" + tagp)
            nc.vector.tensor_mul(hh, x_in, rstd_b)
            nc.vector.tensor_sub(out_bf, hh, mr_b)

        def step(h_in, c_in, h_out, c_out, roff, ooff):
            """One timestep.  roff: row offset (t*BL) into kv;  ooff: out col offset."""
            # LN1
            hn = sp.tile([128, CH, BL], BF16, tag="hn")
            layer_norm(h_in, hn, "1")
            # W1: k, then q (bias first in its group), then v
            pw1 = psB.tile([128, 18, BL], F32, tag="pbig", bufs=2)
            def w1_group(m, bias_row=None):
                if bias_row is not None:
                    nc.tensor.matmul(pw1[:, m, :], lhsT=bias_row[:, m * 128:(m + 1) * 128],
                                     rhs=ones13, start=True, stop=False)
                for c in range(CH):
                    nc.tensor.matmul(pw1[:, m, :], lhsT=w1s[:, c, m * 128:(m + 1) * 128],
                                     rhs=hn[:, c, :], start=(c == 0 and bias_row is None),
                                     stop=(c == CH - 1))
            for m in range(6, 12):
                w1_group(m)
            for m in range(6):
                w1_group(m, qbias)
            for m in range(12, 18):
                w1_group(m)
            # attention
            ke = kv[:, 0, :, ds(roff, BL)]
            ve = kv[:, 1, :, ds(roff, BL)]
            d_bf = sp.tile([128, CH, BL], BF16, tag="dbf")
            nc.vector.tensor_sub(d_bf, pw1[:, 6:12, :], ke)
            m_bf = sp.tile([128, CH, BL], BF16, tag="mbf")
            nc.vector.tensor_mul(m_bf, pw1[:, 0:6, :], d_bf)
            s_ps = psS.tile([NH, BL], F32, tag="sps", bufs=1)
            for c in range(CH):
                nc.tensor.matmul(s_ps, lhsT=maskS[:, c, :], rhs=m_bf[:, c, :],
                                 start=(c == 0), stop=(c == CH - 1))
            p_bf = sp.tile([NH, BL], BF16, tag="pbf")
            nc.scalar.activation(p_bf, s_ps, mybir.ActivationFunctionType.Sigmoid,
                                 scale=float(SCALE))
            pb = psS.tile([128, CH, BL], F32, tag="pb", bufs=1)
            for c in range(CH):
                nc.tensor.matmul(pb[:, c, :], lhsT=maskB[:, c * 128:(c + 1) * 128],
                                 rhs=p_bf, start=True, stop=True)
            u = sp.tile([128, CH, BL], F32, tag="u")
            nc.vector.tensor_sub(u, pw1[:, 12:18, :], ve)
            cx = sp.tile([128, CH, BL], F32, tag="cx")
            nc.vector.tensor_mul(cx, pb, u)
            ctx = sp.tile([128, CH, BL], BF16, tag="ctx")
            nc.vector.tensor_add(ctx, cx, ve)
            # Wc
            pc = psB.tile([128, CH, BL], F32, tag="pbig", bufs=2)
            for m in range(CH):
                nc.tensor.matmul(pc[:, m, :], lhsT=cbias[:, m * 128:(m + 1) * 128],
                                 rhs=ones13, start=True, stop=False)
                for c in range(CH):
                    nc.tensor.matmul(pc[:, m, :], lhsT=wcs[:, c, m * 128:(m + 1) * 128],
                                     rhs=ctx[:, c, :], start=False, stop=(c == CH - 1))
            hx = sp.tile([128, 2, CH, BL], F32, tag="hx")
            nc.vector.tensor_add(hx[:, 0], h_in[:, 0], pc)
            # LN2
            xn = sp.tile([128, CH, BL], BF16, tag="xn")
            layer_norm(hx, xn, "2")
            # gates
            pg = psB.tile([128, 4 * CH, BL], F32, tag="pbig", bufs=2)
            for m in range(4 * CH):
                for c in range(CH):
                    nc.tensor.matmul(pg[:, m, :], lhsT=whhs[:, c, m * 128:(m + 1) * 128],
                                     rhs=xn[:, c, :], start=(c == 0), stop=(c == CH - 1))
            gin = sp.tile([128, 4 * CH, BL], F32, tag="gin")
            hb_b = _ap_bcast(hbias, 128, 0, [(1, 4 * CH), (0, BL)])
            nc.vector.tensor_add(gin, pg, hb_b)
            gates = sp.tile([128, 4 * CH, BL], F32, tag="gates")
            nc.scalar.activation(gates, gin, mybir.ActivationFunctionType.Sigmoid)
            # LSTM cell
            fc = sp.tile([128, CH, BL], F32, tag="fc")
            nc.vector.tensor_mul(fc, gates[:, CH:2 * CH, :], c_in)
            ig = sp.tile([128, CH, BL], F32, tag="ig")
            nc.vector.tensor_mul(ig, gates[:, 0:CH, :], gates[:, 2 * CH:3 * CH, :])
            nc.vector.tensor_add(c_out, fc, ig)
            th = sp.tile([128, CH, BL], F32, tag="th")
            nc.scalar.activation(th, c_out, mybir.ActivationFunctionType.Tanh)
            nc.vector.tensor_mul(h_out[:, 0], gates[:, 3 * CH:4 * CH, :], th)
            nc.sync.dma_start(out_d[:, ds(ooff, CH * BL)],
                              h_out[:, 0].rearrange("p c b -> p (c b)"))
            if debug_step is not None and roff == debug_step * BL:
                flat = lambda t_: t_.rearrange("p c b -> p (c b)")
                nc.sync.dma_start(dbg["dbg_hn"][:], flat(hn))
                nc.sync.dma_start(dbg["dbg_d"][:], flat(d_bf))
                nc.sync.dma_start(dbg["dbg_m"][:], flat(m_bf))
                nc.sync.dma_start(dbg["dbg_p"][:], p_bf)
                nc.sync.dma_start(dbg["dbg_ctx"][:], flat(ctx))
                nc.sync.dma_start(dbg["dbg_hx"][:], flat(hx[:, 0]))
                nc.sync.dma_start(dbg["dbg_xn"][:], flat(xn))
                nc.sync.dma_start(dbg["dbg_g"][:], flat(gates))
                nc.sync.dma_start(dbg["dbg_u"][:], flat(u))

        if unroll_static:
            for t in range(T_steps):
                hi, ci, ho, co = (h0, c0, h1, c1) if t % 2 == 0 else (h1, c1, h0, c0)
                step(hi, ci, ho, co, t * BL, t * CH * BL)
        else:
            assert T_steps % 2 == 0

            def scan_loop():
                with tc.For_i(0, rows, 2 * BL, staggered_reset=True,
                              hint_engines=(mybir.EngineType.PE,
                                            mybir.EngineType.DVE,
                                            mybir.EngineType.Activation)) as rr:
                    step(h0, c0, h1, c1, rr, rr * CH)
                    step(h1, c1, h0, c0, rr + BL, rr * CH + CH * BL)

            if passes == 1:
                scan_loop()
            else:
                with tc.For_i(0, passes, 1):
                    nc.vector.memset(h0, 0.0)
                    nc.vector.memset(c0, 0.0)
                    scan_loop()

    nc.finalize()
    return nc


# ---------------------------------------------------------------------------
# host side
# ---------------------------------------------------------------------------

def _ln_np(x, g, b, eps=EPS):
    m = x.mean(-1, keepdims=True)
    v = ((x - m) ** 2).mean(-1, keepdims=True)
    return (x - m) / np.sqrt(v + eps) * g + b


def _normalize_np(x, eps=EPS):
    m = x.mean(-1, keepdims=True)
    v = ((x - m) ** 2).mean(-1, keepdims=True)
    return (x - m) / np.sqrt(v + eps)


def _chunked(w):
    """(768, X) fp32 -> (128, 6*X) bf16 in chunk-major layout."""
    X = w.shape[1]
    return np.ascontiguousarray(
        w.reshape(CH, 128, X).transpose(1, 0, 2).reshape(128, CH * X)
    ).astype(bf16)


def prepare_inputs(input_ids, emb, ln_e_g, ln_e_b, ln1_g, ln1_b, ln2_g, ln2_b,
                   Wkv, bkv, Wq, bq, Wc, bc, Whh, bhh, T_steps=T):
    f = np.float32
    emb = np.asarray(emb, f)
    input_ids = np.asarray(input_ids)
    ln_e_g, ln_e_b = np.asarray(ln_e_g, f), np.asarray(ln_e_b, f)
    ln1_g, ln1_b = np.asarray(ln1_g, f), np.asarray(ln1_b, f)
    ln2_g, ln2_b = np.asarray(ln2_g, f), np.asarray(ln2_b, f)
    Wkv, bkv = np.asarray(Wkv, f), np.asarray(bkv, f)
    Wq, bq = np.asarray(Wq, f), np.asarray(bq, f)
    Wc, bc = np.asarray(Wc, f), np.asarray(bc, f)
    Whh, bhh = np.asarray(Whh, f), np.asarray(bhh, f)

    emb2 = _normalize_np(_ln_np(emb, ln_e_g, ln_e_b))        # (V, H)

    W1f = ln1_g[:, None] * np.hstack([Wq, Wkv])              # (768, 2304) [q|k|v]
    qbias_eff = bq + ln1_b @ Wq                              # (768,)
    kvbias_eff = bkv + ln1_b @ Wkv                           # (1536,)
    vbias_eff = kvbias_eff[H:]                               # (768,)
    WhhTf = (Whh * ln2_g[None, :]).T                         # (768, 3072) [i|f|g|o]
    bhh_f = bhh + Whh @ ln2_b                                # (3072,)

    w1_in = _chunked(W1f)
    wc_in = _chunked(Wc)
    whh_in = _chunked(WhhTf)
    vbias_in = vbias_eff.reshape(1, H).astype(bf16)
    qbias_in = qbias_eff.reshape(1, H).astype(bf16)
    cbias_in = bc.reshape(1, H).astype(bf16)
    hbias_in = np.ascontiguousarray(bhh_f.reshape(4 * CH, 128).T).astype(f)

    p_idx = np.arange(128)
    c_idx = np.arange(CH)
    j_idx = np.arange(NH)
    # maskS[p, c, j] = 1 if j == 2c + p//64
    maskS = (j_idx[None, None, :] == (2 * c_idx[None, :, None] + p_idx[:, None, None] // 64))
    maskS_in = maskS.reshape(128, CH * NH).astype(bf16)
    # maskB[j, c, p] = same predicate
    maskB = (j_idx[:, None, None] == (2 * c_idx[None, :, None] + p_idx[None, None, :] // 64))
    maskB_in = maskB.reshape(NH, CH * 128).astype(bf16)

    ids_pad = np.zeros((NCORES * BL, T), dtype=np.int64)
    ids_pad[:B] = input_ids
    e2 = emb2[ids_pad]                                       # (104, T, H) f32

    in_maps = []
    for k in range(NCORES):
        sl = e2[k * BL:(k + 1) * BL, :T_steps, :]            # (13, Ts, 768)
        x = sl.transpose(2, 1, 0).reshape(H, T_steps * BL)   # (768, rows)
        e2T_in = np.ascontiguousarray(
            x.reshape(CH, 128, T_steps * BL).transpose(1, 0, 2)
            .reshape(128, CH * T_steps * BL)
        ).astype(bf16)
        in_maps.append({
            "e2T": e2T_in, "w1": w1_in, "wc": wc_in, "whh": whh_in,
            "vbias": vbias_in, "qbias": qbias_in, "cbias": cbias_in,
            "hbias": hbias_in, "maskS": maskS_in, "maskB": maskB_in,
        })
    return in_maps


def assemble_output(results, T_steps=T):
    out = np.empty((B, T_steps, H), dtype=np.float32)
    for k in range(NCORES):
        arr = results[k]["out"]                              # (128, Ts*CH*BL)
        o = arr.reshape(128, T_steps, CH, BL).transpose(3, 1, 2, 0).reshape(BL, T_steps, H)
        lo = k * BL
        hi = min(B, lo + BL)
        if hi > lo:
            out[lo:hi] = o[:hi - lo]
    return out


def kernel(**inputs):
    in_maps = prepare_inputs(**inputs)
    nc = build_bass(T)
    res = run_bass_kernel_spmd(nc, in_maps, core_ids=list(range(NCORES)))
    return assemble_output(res.results)


if __name__ == "__main__":
    nc = build_bass(T)
    print("built ok")
